# revision 13
# baseline (speedup 1.0000x reference)
"""AdaptiveLowPassFilter Trainium2 kernel — 8-core batch-parallel.

Per core (one image, x [96,128,128] f32):
  phase A  (PE):  fused depthwise3x3+pw1 as 9 shifted matmuls accumulating in
                  PSUM (channels on partitions, padded flat pixels free; all
                  9 taps are pure free-dim offsets).
  leaky    (ACT): bias + LeakyReLU(0.2) from PSUM into h2 (bf16).
  phase B  (PE):  pw2 computed TRANSPOSED per image row: lhsT=h2[:,row],
                  rhs=pw2^T (aug with bias row) -> logits land pixel-major
                  [w, 9] in PSUM; ACT exp -> unnormalized e_t.
  softmax  (DVE): den = reduce over 9 taps, reciprocal; kw2 = e*recip written
                  as duplicated PAIRS (so phase C multiplies hit the DVE
                  2x_1P mode: innermost AP dim is a step-1 bf16 pair).
  x_t      (PE):  per-row transposes of x into pixel-major [w, (h, c)];
                  w+-1 taps via partition-shifted SBUF->SBUF band DMAs.
  phase C  (DVE): per 16-row band: 9 big tensor_tensor pair-multiplies +
                  8 big adds: num[w,(h,c)] = sum_k kw[w,h,k]*x_t{shift}[...].
  out      (ACT): bf16->f32, DMA out as [w, (h, c)]; host transposes.

All phases are banded (16 rows) so Tile can pipeline engines end-to-end.
"""
import sys
sys.path.insert(0, "/opt/trn_rl_repo")

import numpy as np
import ml_dtypes
from contextlib import ExitStack

C, CO, H, W, K = 96, 48, 128, 128, 9
RS = 130            # padded row stride in flat pixel space
PIX0 = 131          # flat offset of pixel (0, 0)
XBF = 17160         # x_bf free size
NQ = 16896          # 33 chunks x 512 of h2 pixel space
NCHUNK = 33
CHUNK = 512
HB = 16             # band height (rows)
NB = H // HB        # 8 bands

_CACHE = {}


def _build():
    import concourse.bass as bass
    import concourse.bacc as bacc
    import concourse.tile as tile
    import concourse.mybir as mybir

    dt = mybir.dt
    f32, bf16 = dt.float32, dt.bfloat16
    AF = mybir.ActivationFunctionType
    OP = mybir.AluOpType

    nc = bacc.Bacc("TRN2", target_bir_lowering=False, debug=False)
    x_d = nc.dram_tensor("x", (C, H, W), f32, kind="ExternalInput")
    wk_d = nc.dram_tensor("wk", (C, K * CO), bf16, kind="ExternalInput")
    pw2t_d = nc.dram_tensor("pw2t", (CO + 1, K), bf16, kind="ExternalInput")
    bh2_d = nc.dram_tensor("bh2", (CO, 1), f32, kind="ExternalInput")
    iden_d = nc.dram_tensor("iden", (C, C), bf16, kind="ExternalInput")
    ones_d = nc.dram_tensor("ones", (1, NQ), bf16, kind="ExternalInput")
    y_d = nc.dram_tensor("y", (W, H * C), f32, kind="ExternalOutput")

    with ExitStack() as ctx:
        tc = ctx.enter_context(tile.TileContext(nc))
        st = ctx.enter_context(tc.tile_pool(name="st", bufs=1))
        xsp = ctx.enter_context(tc.tile_pool(name="xsp", bufs=2))
        prp = ctx.enter_context(tc.tile_pool(name="prp", bufs=2))
        nump = ctx.enter_context(tc.tile_pool(name="nump", bufs=2))
        outp = ctx.enter_context(tc.tile_pool(name="outp", bufs=2))
        h2p = ctx.enter_context(tc.tile_pool(name="h2p", bufs=3, space="PSUM"))
        ltp = ctx.enter_context(tc.tile_pool(name="ltp", bufs=2, space="PSUM"))
        xtp = ctx.enter_context(tc.tile_pool(name="xtp", bufs=3, space="PSUM"))

        x_bf = st.tile([C, XBF], bf16, tag="x_bf")
        h2a = st.tile([CO + 1, NQ], bf16, tag="h2a")
        e_t = st.tile([W, H * K], bf16, tag="e_t")
        kw2 = st.tile([W, H * K * 2], bf16, tag="kw2")   # normalized, dup pairs
        den = st.tile([W, H], f32, tag="den")
        recip = st.tile([W, H], f32, tag="recip")
        x_t = st.tile([W, RS * C], bf16, tag="x_t")
        x_tm1 = st.tile([W, RS * C], bf16, tag="x_tm1")
        x_tp1 = st.tile([W, RS * C], bf16, tag="x_tp1")
        wk_sb = st.tile([C, K * CO], bf16, tag="wk_sb")
        pw2t_sb = st.tile([CO + 1, K], bf16, tag="pw2t_sb")
        bh2_sb = st.tile([CO, 1], f32, tag="bh2_sb")
        iden_sb = st.tile([C, C], bf16, tag="iden_sb")

        # ---- init: params + pad memsets
        nc.sync.dma_start(wk_sb[:], wk_d.ap())
        nc.sync.dma_start(pw2t_sb[:], pw2t_d.ap())
        nc.sync.dma_start(bh2_sb[:], bh2_d.ap())
        nc.sync.dma_start(iden_sb[:], iden_d.ap())
        nc.sync.dma_start(h2a[CO:CO + 1, :], ones_d.ap())
        # x_bf zero pads: head, per-row 2-col gaps, tail
        nc.gpsimd.memset(x_bf[:, 0:PIX0], 0.0)
        nc.gpsimd.memset(
            x_bf[:, PIX0 + W:PIX0 + W + 127 * RS]
            .rearrange("p (g t) -> p g t", t=RS)[:, :, 0:2], 0.0)
        nc.gpsimd.memset(x_bf[:, PIX0 + 127 * RS + W:XBF], 0.0)
        # x_t / shifts: zero pad row-slots 0 and 129; zero edge partitions
        for t in (x_t, x_tm1, x_tp1):
            nc.gpsimd.memset(t[:, 0:C], 0.0)
            nc.gpsimd.memset(t[:, (RS - 1) * C:RS * C], 0.0)

        # ---- 3-stage band pipeline: front(b) | mid(b-1) | back(b-2)
        def front(b):
            h0 = b * HB
            for g2 in range(2):
                hh = h0 + g2 * 8
                xs = xsp.tile([C, 8 * W], f32, tag="xs")
                nc.sync.dma_start(xs[:], x_d.ap()[:, hh:hh + 8, :])
                dst = (x_bf[:, PIX0 + hh * RS: PIX0 + (hh + 8) * RS]
                       .rearrange("p (h w) -> p h w", w=RS)[:, :, 0:W])
                nc.scalar.copy(dst, xs[:].rearrange("p (h w) -> p h w", w=W))
            # transposes: 2 groups of 8 rows x 3 shifts -> psum -> x_t*
            # (w+-1 taps = free-dim offset of the transpose input; padded
            #  gap columns supply the correct zeros at row edges)
            for g2 in range(2):
                hh0 = h0 + g2 * 8
                for dst, dq in ((x_tm1, -1), (x_t, 0), (x_tp1, 1)):
                    xt_ps = xtp.tile([W, 8 * C], bf16, tag="xt_ps")
                    for r in range(8):
                        h = hh0 + r
                        q = PIX0 + h * RS + dq
                        nc.tensor.transpose(
                            xt_ps[:, r * C:(r + 1) * C],
                            x_bf[:, q:q + W],
                            iden_sb[:],
                        )
                    nc.scalar.copy(
                        dst[:, (hh0 + 1) * C:(hh0 + 9) * C], xt_ps[:])

        def mid(b):
            h0 = b * HB
            # phase A chunks owned by this band, in groups of 3 so each
            # tap's weights are loaded once per group (amortize LDWEIGHTS)
            own = [i for i in range(NCHUNK) if (CHUNK * i) // (RS * HB) == b]
            for g0 in range(0, len(own), 3):
                grp = own[g0:g0 + 3]
                pss = []
                for _pi in range(len(grp)):
                    ps_t = h2p.tile([CO, CHUNK], f32, tag="h2ps")
                    pss.append(ps_t)
                for k in range(K):
                    delta = (k // 3 - 1) * RS + (k % 3 - 1)
                    for ps, i in zip(pss, grp):
                        q0 = PIX0 + CHUNK * i
                        nc.tensor.matmul(
                            ps[:],
                            lhsT=wk_sb[:, k * CO:(k + 1) * CO],
                            rhs=x_bf[:, q0 + delta:q0 + delta + CHUNK],
                            start=(k == 0), stop=(k == K - 1),
                        )
                for ps, i in zip(pss, grp):
                    nc.scalar.activation(
                        h2a[0:CO, CHUNK * i:CHUNK * (i + 1)], ps[:],
                        AF.Lrelu, bias=bh2_sb[:], scale=1.0, alpha=0.2,
                    )
            # phase B: transposed pw2 -> pixel-major logits psum
            lt = ltp.tile([W, HB * K], f32, tag="lt")
            for r in range(HB):
                h = h0 + r
                nc.tensor.matmul(
                    lt[:, r * K:(r + 1) * K],
                    lhsT=h2a[:, h * RS:h * RS + W],
                    rhs=pw2t_sb[:],
                    start=True, stop=True,
                )
            eb = e_t[:, h0 * K:(h0 + HB) * K]
            nc.scalar.activation(eb, lt[:], AF.Exp)
            # den, recip, kw2 (normalized dup-pairs) for this band
            db = den[:, h0:h0 + HB]
            nc.vector.tensor_reduce(
                db, eb.rearrange("p (h k) -> p h k", k=K),
                axis=mybir.AxisListType.X, op=OP.add)
            rb = recip[:, h0:h0 + HB]
            nc.vector.reciprocal(rb, db)
            nc.vector.tensor_mul(
                kw2[:, h0 * K * 2:(h0 + HB) * K * 2]
                .rearrange("p (h k d) -> p h k d", k=K, d=2),
                eb.rearrange("p (h k) -> p h k", k=K)
                .unsqueeze(3).broadcast_to([W, HB, K, 2]),
                rb.unsqueeze(2).broadcast_to([W, HB, K])
                .unsqueeze(3).broadcast_to([W, HB, K, 2]),
            )

        srcs = {0: x_tm1, 1: x_t, 2: x_tp1}

        def back(b):
            h0 = b * HB
            # phase C: 9 pair-multiplies + 8 adds on [W, HB*C]
            numt = nump.tile([W, HB * C], bf16, tag="numt")
            accv = numt[:]
            for k in range(K):
                i, j = k // 3, k % 3
                xsrc = (srcs[j][:, (h0 + i) * C:(h0 + i + HB) * C]
                        .rearrange("p (h c2 d) -> p h c2 d", c2=C // 2, d=2))
                kwv = (kw2[:, h0 * K * 2:(h0 + HB) * K * 2]
                       .rearrange("p (h k d) -> p h k d", k=K, d=2)[:, :, k, :]
                       .unsqueeze(2).broadcast_to([W, HB, C // 2, 2]))
                if k == 0:
                    nc.vector.tensor_tensor(
                        accv.rearrange("p (h c2 d) -> p h c2 d", c2=C // 2, d=2),
                        xsrc, kwv, op=OP.mult)
                else:
                    prod = prp.tile([W, HB * C], bf16, tag="prod")
                    nc.vector.tensor_tensor(
                        prod[:].rearrange("p (h c2 d) -> p h c2 d", c2=C // 2, d=2),
                        xsrc, kwv, op=OP.mult)
                    nc.vector.tensor_add(accv, accv, prod[:])
            # out: convert + store
            ot = outp.tile([W, HB * C], f32, tag="ot")
            nc.scalar.copy(ot[:], accv)
            nc.sync.dma_start(y_d.ap()[:, h0 * C:(h0 + HB) * C], ot[:])

        for b in range(NB + 2):
            if b < NB:
                front(b)
            if 1 <= b <= NB:
                mid(b - 1)
            if b >= 2:
                back(b - 2)

    nc.compile()
    return nc


def _get_nc():
    if "nc" not in _CACHE:
        _CACHE["nc"] = _build()
    return _CACHE["nc"]


def kernel(x, dw_w, dw_b, pw1_w, pw1_b, pw2_w, pw2_b):
    from concourse.bass_utils import run_bass_kernel_spmd

    x = np.asarray(x, np.float32)
    dw_w = np.asarray(dw_w, np.float32)
    dw_b = np.asarray(dw_b, np.float32)
    pw1_w = np.asarray(pw1_w, np.float32)
    pw1_b = np.asarray(pw1_b, np.float32)
    pw2_w = np.asarray(pw2_w, np.float32)
    pw2_b = np.asarray(pw2_b, np.float32)

    bf = ml_dtypes.bfloat16
    # fused weights: wk[c, k*CO + o] = pw1_w[o, c] * dw_w[c, 0, k//3, k%3]
    wk = np.empty((C, K, CO), np.float32)
    for k in range(K):
        wk[:, k, :] = pw1_w.T * dw_w[:, 0, k // 3, k % 3][:, None]
    wk = wk.reshape(C, K * CO).astype(bf)
    pw2t = np.concatenate([pw2_w.T, pw2_b[None, :]], axis=0).astype(bf)
    bh2 = (pw1_w @ dw_b + pw1_b).reshape(CO, 1).astype(np.float32)
    iden = np.eye(C, dtype=np.float32).astype(bf)
    ones = np.ones((1, NQ), np.float32).astype(bf)

    nc = _get_nc()
    in_maps = [
        {"x": np.ascontiguousarray(x[b]), "wk": wk, "pw2t": pw2t,
         "bh2": bh2, "iden": iden, "ones": ones}
        for b in range(8)
    ]
    res = run_bass_kernel_spmd(nc, in_maps, core_ids=list(range(8)),
                               **_CACHE.get("run_kwargs", {}))
    _CACHE["last_result"] = res
    out = np.empty((8, C, H, W), np.float32)
    for b in range(8):
        out[b] = res.results[b]["y"].reshape(W, H, C).transpose(2, 1, 0)
    return out
